# revision 35
# baseline (speedup 1.0000x reference)
"""GQA attention (SEQ=2048, DIM=4096, 32 Q heads / 8 KV heads, head_dim=128),
tensor-parallel over heads across 8 NeuronCores.

Each core owns 4 Q heads + 1 KV head: wq/wk/wv split column-wise, wo split
row-wise; each core produces a partial (2048, 4096) output that the host sums
(the all-reduce of row-parallel wo).

Per-core kernel, bf16 matmul operands (1 cyc/row at any free size, half the
DMA/SBUF/DVE traffic of f32; PSUM accumulation stays fp32):
  A) QKV projections over per-chunk x tiles: an interleaved K||V pass tracks
     the x DMA stream, then Q0..Q3 passes re-read the resident chunks while
     RoPE / V-transpose work drains on ACT+DVE+PE underneath. Weight DMAs
     ride the (otherwise idle) GPSIMD queue so the x stream owns the sync
     queue's DMA engine.
  B/C) software-pipelined: the attention streams for query block qb carry
     the output projection of block qb-1 inside them. Per (head h, qb):
     S^T matmuls + exp (ACT, scale folded) + in-place 128x128 0/1 triangle
     mask on the diagonal (DVE) + row-sum accumulation (split GPSIMD/DVE)
     + lagged AV drains, with the wo-projection pair groups of (qb-1, qc=h)
     and the previous head's deferred tail drains / D-reciprocal chain
     interleaved between score matmuls. Diagonal blocks shrink their moving
     free dim to the causally-visible suffix. D is broadcast across
     partitions via a ones-matrix matmul, inverted with the fast custom-DVE
     reciprocal, and applied to O^T on DVE.
"""

import numpy as np

import concourse.bacc as bacc
import concourse.tile as tile
from concourse import mybir
from concourse.bass_utils import run_bass_kernel_spmd

F32 = mybir.dt.float32
F32R = mybir.dt.float32r
BF16 = mybir.dt.bfloat16

DIM = 4096
SEQ = 2048
HEAD_DIM = 128
N_CORES = 8
QH = 4              # q heads per core
QS = QH * HEAD_DIM  # 512: wq column slice per core
NKT = DIM // 128    # 32 contraction tiles
NSB = SEQ // 512    # 4 sequence blocks
SCALE = 1.0 / float(np.sqrt(HEAD_DIM))
LAG = 4             # AV matmuls trail the score stream by LAG blocks


def build_nc():
    nc = bacc.Bacc(trn_type="TRN2")

    # all big operands are pre-shuffled on the host into partition-major
    # layouts so every DMA moves 4-8 KiB contiguous lines per partition
    xS = nc.declare_dram_parameter("xS", [128, NSB, NKT, 512], BF16, isOutput=False)
    wqS = nc.declare_dram_parameter("wqS", [128, NKT * QS], BF16, isOutput=False)
    wkS = nc.declare_dram_parameter("wkS", [128, NKT * HEAD_DIM], BF16, isOutput=False)
    wvS = nc.declare_dram_parameter("wvS", [128, NKT * HEAD_DIM], BF16, isOutput=False)
    woS = nc.declare_dram_parameter("woS", [128, QH * DIM], BF16, isOutput=False)
    cosT = nc.declare_dram_parameter("cosT", [HEAD_DIM, SEQ], BF16, isOutput=False)
    sinTs = nc.declare_dram_parameter("sinTs", [HEAD_DIM, SEQ], BF16, isOutput=False)
    tri = nc.declare_dram_parameter("tri", [128, 128], BF16, isOutput=False)
    ident = nc.declare_dram_parameter("ident", [128, 128], BF16, isOutput=False)
    ones_mat = nc.declare_dram_parameter("ones_mat", [128, 128], F32R, isOutput=False)
    out = nc.declare_dram_parameter("out", [SEQ, DIM], BF16, isOutput=True)

    with tile.TileContext(nc) as tc:
        with (
            tc.tile_pool(name="persist", bufs=1) as persist,
            tc.tile_pool(name="resid", bufs=1) as resid,
        ):
            tri_sb = persist.tile([128, 128], BF16)
            ident_sb = persist.tile([128, 128], BF16)
            ones_sb = persist.tile([128, 128], F32R)

            # resident activations; qT split per head so phase B's first
            # score matmuls only wait on their own head's RoPE
            qTs = [resid.tile([128, SEQ], BF16, name=f"qT{h}")
                   for h in range(QH)]
            kT = resid.tile([128, SEQ], BF16)              # K^T (d, s)
            vN = resid.tile([128, SEQ // 128, 128], BF16)  # V natural (k, d)

            # ---------------- Phase A: projections + RoPE ----------------
            with (
                tc.tile_pool(name="wpool", bufs=1) as wpool,
                tc.tile_pool(name="xpool", bufs=2) as xpool,
                tc.tile_pool(name="cspool", bufs=2) as cspool,
                tc.tile_pool(name="ropetmp", bufs=2) as ropetmp,
                tc.tile_pool(name="vtb", bufs=2) as vtb,
                tc.tile_pool(name="psA", bufs=1, space="PSUM") as psA,
                tc.tile_pool(name="psVT", bufs=2, space="PSUM") as psVT,
            ):
                # per-chunk weight tiles: a matmul only waits on the one DMA
                # that feeds its chunk, not the whole weight load. Weights
                # ride the scalar HW-DGE queue, x owns the sync queue.
                wk_cs = [wpool.tile([128, 8 * HEAD_DIM], BF16, name=f"wk{c}")
                         for c in range(4)]
                wv_cs = [wpool.tile([128, 8 * HEAD_DIM], BF16, name=f"wv{c}")
                         for c in range(4)]
                wq_cs = [wpool.tile([128, 4 * QS], BF16, name=f"wq{c}")
                         for c in range(8)]

                def xchunks(sb):
                    xs = []
                    for g in range(8):
                        xg = xpool.tile([128, 4, 512], BF16, tag=f"xb{g}",
                                        name=f"xb{g}")
                        nc.sync.dma_start(out=xg, in_=xS[:, sb, g * 4:(g + 1) * 4, :])
                        xs.append(xg)
                    return xs

                def wdma(c, q):
                    nc.scalar.dma_start(
                        out=wk_cs[c], in_=wkS[:, c * 1024:(c + 1) * 1024])
                    nc.scalar.dma_start(
                        out=wv_cs[c], in_=wvS[:, c * 1024:(c + 1) * 1024])
                    for i in range(2):
                        if 2 * c + i in q:
                            nc.scalar.dma_start(
                                out=wq_cs[2 * c + i],
                                in_=wqS[:, (2 * c + i) * 2048:
                                        (2 * c + i + 1) * 2048])

                # scalar queue in exact consumption order; the last wq
                # chunks ride the sync queue once the x stream is done
                wdma(0, (0, 1))
                xb = xchunks(0)
                wdma(1, (2, 3))
                cos_t = cspool.tile([128, 512], BF16, tag="cos")
                nc.scalar.dma_start(out=cos_t, in_=cosT[:, 0:512])
                sin_t = cspool.tile([128, 512], BF16, tag="sin")
                nc.scalar.dma_start(out=sin_t, in_=sinTs[:, 0:512])
                wdma(2, (4,))
                wdma(3, ())
                for i in (5, 6, 7):
                    nc.sync.dma_start(
                        out=wq_cs[i],
                        in_=wqS[:, i * 2048:(i + 1) * 2048])

                # small constants ride behind the critical weight DMAs
                nc.scalar.dma_start(out=ident_sb, in_=ident[:, :])
                nc.scalar.dma_start(out=tri_sb, in_=tri[:, :])
                nc.scalar.dma_start(out=ones_sb, in_=ones_mat[:, :])

                def rope(dst, src_ps, cos_t, sin_t):
                    # ACT copies evict PSUM fast and produce the straight and
                    # half-rotated views (partition-shifted reads are only
                    # legal on ACT); DVE runs the bf16 mul/mul/add at 2x rate.
                    v = ropetmp.tile([128, 512], BF16, tag="v", name="v")
                    vr = ropetmp.tile([128, 512], BF16, tag="vr", name="vr")
                    nc.scalar.copy(v, src_ps)
                    nc.scalar.copy(vr[0:64, :], src_ps[64:128, :])
                    nc.scalar.copy(vr[64:128, :], src_ps[0:64, :])
                    t = ropetmp.tile([128, 512], BF16, tag="t", name="t")
                    u = ropetmp.tile([128, 512], BF16, tag="u", name="u")
                    nc.vector.tensor_mul(t, v, cos_t)
                    nc.vector.tensor_mul(u, vr, sin_t)
                    nc.vector.tensor_add(dst, t, u)

                for sb in range(NSB):
                    ss = slice(sb * 512, (sb + 1) * 512)
                    if sb > 0:
                        xb = xchunks(sb)
                        cos_t = cspool.tile([128, 512], BF16, tag="cos")
                        nc.scalar.dma_start(out=cos_t, in_=cosT[:, ss])
                        sin_t = cspool.tile([128, 512], BF16, tag="sin")
                        nc.scalar.dma_start(out=sin_t, in_=sinTs[:, ss])

                    # interleaved K||V(||Q0 for the cold-start block)
                    # pass: the PE consumes each x chunk at the DMA delivery
                    # rate; rope-K and the V evict then run on ACT/DVE under
                    # the remaining Q passes
                    def qmm(q_ps, h, kt):
                        nc.tensor.matmul(
                            q_ps,
                            wq_cs[kt // 4][:, (kt % 4) * 512 + h * 128:
                                           (kt % 4) * 512 + (h + 1) * 128],
                            xb[kt // 4][:, kt % 4, :],
                            start=(kt == 0), stop=(kt == NKT - 1),
                        )

                    k_ps = psA.tile([128, 512], F32, tag="kps", name="kps")
                    v_ps = psA.tile([128, 512], F32, tag="vps", name="vps")
                    q_ps0 = psA.tile([128, 512], F32, tag="qps0", name="qps0")

                    def kmm(kt):
                        nc.tensor.matmul(
                            k_ps,
                            wk_cs[kt // 8][:, (kt % 8) * 128:
                                           (kt % 8 + 1) * 128],
                            xb[kt // 4][:, kt % 4, :],
                            start=(kt == 0), stop=(kt == NKT - 1),
                        )

                    def vmm(kt):
                        nc.tensor.matmul(
                            v_ps,
                            wv_cs[kt // 8][:, (kt % 8) * 128:
                                           (kt % 8 + 1) * 128],
                            xb[kt // 4][:, kt % 4, :],
                            start=(kt == 0), stop=(kt == NKT - 1),
                        )

                    for kt in range(NKT):
                        kmm(kt)
                        vmm(kt)
                        if sb == 0:
                            qmm(q_ps0, 0, kt)
                    rope(kT[:, ss], k_ps, cos_t, sin_t)
                    vt_sb = vtb.tile([128, 512], BF16, tag="vt", name="vt")
                    nc.scalar.copy(vt_sb, v_ps)
                    if sb == 0:
                        rope(qTs[0][:, ss], q_ps0, cos_t, sin_t)

                    for h in range(QH):
                        if h == 0:
                            if sb == 0:
                                continue
                            q_ps = q_ps0
                        else:
                            q_ps = psA.tile([128, 512], F32, tag=f"qps{h}",
                                            name=f"qps{h}")
                        for kt in range(NKT):
                            qmm(q_ps, h, kt)
                        if h == 1:
                            # PE transposes of V^T -> V, slotted between Q
                            # passes so they never wait on the ACT evict
                            for j in range(4):
                                vt_ps = psVT.tile([128, 128], BF16, tag="vtp",
                                                  name="vtp")
                                nc.tensor.transpose(
                                    vt_ps, vt_sb[:, j * 128:(j + 1) * 128],
                                    ident_sb,
                                )
                                nc.scalar.copy(vN[:, sb * 4 + j, :], vt_ps)
                        rope(qTs[h][:, ss], q_ps, cos_t, sin_t)

            # ------- Phase B/C: attention with pipelined out projection -------
            with (
                tc.tile_pool(name="wopool", bufs=1) as wopool,
                tc.tile_pool(name="expp", bufs=24) as expp,
                tc.tile_pool(name="esum", bufs=2) as esum,
                tc.tile_pool(name="rdp", bufs=2) as rdp,
                tc.tile_pool(name="otp", bufs=2) as otp,
                tc.tile_pool(name="stg", bufs=2) as stg,
                # declaration order maps pools onto the banks phase A frees
                # first (k/v accumulators release during the Q passes; the
                # q accumulators only after their trailing RoPE)
                tc.tile_pool(name="psOT", bufs=2, space="PSUM") as psOT,
                tc.tile_pool(name="psS", bufs=2, space="PSUM") as psS,
                tc.tile_pool(name="psD", bufs=2, space="PSUM") as psD,
                tc.tile_pool(name="psC", bufs=2, space="PSUM") as psC,
            ):
                wo_sbs = [wopool.tile([128, DIM], BF16, name=f"wo{h}")
                          for h in range(QH)]
                for h in range(QH):
                    nc.scalar.dma_start(
                        out=wo_sbs[h], in_=woS[:, h * DIM:(h + 1) * DIM]
                    )

                ot_store = {}   # (qb, h) -> normalized O^T tile
                deferred = []   # closures: tail drains + D chains, issued
                                # under later PE work

                def flush(n=None):
                    k = len(deferred) if n is None else min(n, len(deferred))
                    for _ in range(k):
                        deferred.pop(0)()

                def c_items(qb, qc, last=False):
                    """Output-projection work of query block qb, row stripe
                    qc: 8 accumulation groups + evicts + 2 half-stripe DMAs,
                    as a list of closures."""
                    stg_t = stg.tile([128, DIM], BF16, tag="stg", name="stg")
                    items = []

                    def group(nb, evict_dve):
                        def go():
                            o_ps = psC.tile([128, 512], F32, tag="ops",
                                            name="ops")
                            for h in range(QH):
                                nc.tensor.matmul(
                                    o_ps,
                                    ot_store[(qb, h)][:, qc * 128:
                                                      (qc + 1) * 128],
                                    wo_sbs[h][:, nb * 512:(nb + 1) * 512],
                                    start=(h == 0), stop=(h == QH - 1),
                                )
                            dst = stg_t[:, nb * 512:(nb + 1) * 512]
                            if evict_dve:
                                nc.vector.tensor_copy(dst, o_ps)
                            else:
                                nc.scalar.copy(dst, o_ps)
                            if last:
                                nc.sync.dma_start(
                                    out=out[qb * 512 + qc * 128:
                                            qb * 512 + (qc + 1) * 128,
                                            nb * 512:(nb + 1) * 512],
                                    in_=dst,
                                )
                        return go

                    def dma(lo, hi):
                        def go():
                            nc.sync.dma_start(
                                out=out[qb * 512 + qc * 128:
                                        qb * 512 + (qc + 1) * 128,
                                        lo * 512:hi * 512],
                                in_=stg_t[:, lo * 512:hi * 512],
                            )
                        return go

                    for nb in range(8):
                        # during-stream evicts lean on DVE (ACT is running
                        # the exp stream); the trailing block alternates
                        items.append(group(nb, evict_dve=(nb < 5) if not last
                                           else nb % 2 == 0))
                        if not last:
                            if nb == 3:
                                items.append(dma(0, 4))
                            elif nb == 7:
                                items.append(dma(4, 8))
                    return items

                for qb in range(NSB):
                    qs = slice(qb * 512, (qb + 1) * 512)
                    n_kb = 4 * qb + 4
                    for h in range(QH):
                        citems = c_items(qb - 1, h) if qb > 0 else []
                        c_done = 0
                        ot_ps = psOT.tile([128, 512], F32, tag="otps",
                                          name="otps")
                        # row-sum accumulators: [tile, start offset or None,
                        # engine] -- one fed by GPSIMD, one by DVE
                        accs = [
                            [esum.tile([128, 512], F32R, tag="esa",
                                       name="esa"), None,
                             nc.vector if h == QH - 1 else nc.gpsimd],
                            [esum.tile([128, 512], F32R, tag="esb",
                                       name="esb"), None, nc.vector],
                        ]
                        nacc = [0]
                        pend = [None]

                        def acc_push(t, off):
                            a = accs[nacc[0] % 2]
                            nacc[0] += 1
                            if a[1] is None:
                                a[2].tensor_copy(a[0][:, off:], t[:, off:])
                                a[1] = off
                            else:
                                a[2].tensor_add(a[0][:, off:],
                                                a[0][:, off:], t[:, off:])
                        ess = [None] * n_kb
                        offs = [max(0, kb - 4 * qb) * 128
                                for kb in range(n_kb)]

                        def drain(kb, ot_ps=ot_ps, ess=ess, n_kb=n_kb,
                                  offs=offs):
                            o = offs[kb]
                            nc.tensor.matmul(
                                ot_ps[:, o:], vN[:, kb, :], ess[kb][:, o:],
                                start=(kb == 0), stop=(kb == n_kb - 1),
                            )

                        for kb in range(n_kb):
                            off = offs[kb]
                            s_ps = psS.tile([128, 512], F32, tag="sps",
                                            name="sps")
                            nc.tensor.matmul(
                                s_ps[:, off:],
                                kT[:, kb * 128:(kb + 1) * 128],
                                qTs[h][:, qb * 512 + off:(qb + 1) * 512],
                                start=True, stop=True,
                            )
                            es = expp.tile([128, 512], BF16, tag="es",
                                           name="es")
                            nc.scalar.activation(
                                es[:, off:], s_ps[:, off:],
                                mybir.ActivationFunctionType.Exp,
                                scale=SCALE,
                            )
                            if kb - 4 * qb >= 0:
                                # in-place 0/1 lower-triangle mask on the
                                # 128-wide diagonal sub-block
                                nc.vector.tensor_mul(
                                    es[:, off:off + 128],
                                    es[:, off:off + 128], tri_sb,
                                )
                            ess[kb] = es
                            # row-sum: full-width blocks reduce pairwise in
                            # bf16 on the DVE 2x path first (error averages
                            # out over the pair), then the f32 accumulators
                            # alternate GPSIMD/DVE
                            if off == 0 and kb < 4 * qb:
                                if pend[0] is None:
                                    pend[0] = es
                                else:
                                    esp = expp.tile([128, 512], BF16,
                                                    tag="es", name="esp")
                                    nc.vector.tensor_add(esp, pend[0], es)
                                    pend[0] = None
                                    acc_push(esp, 0)
                            else:
                                acc_push(es, off)
                            if kb >= LAG:
                                drain(kb - LAG)
                            # previous head's deferred chain, then this
                            # head's share of the qb-1 projection work
                            if kb == 0:
                                flush(2)
                            elif kb == 1:
                                flush(2)
                            elif kb == 2:
                                flush()
                            elif citems:
                                hold = 8 if h == QH - 1 else 0
                                want = ((len(citems) - hold) * (kb - 2)
                                        ) // (n_kb - 3)
                                while c_done < want:
                                    citems[c_done]()
                                    c_done += 1
                        for kb in range(max(0, n_kb - LAG), n_kb):
                            deferred.append(
                                lambda kb=kb, drain=drain: drain(kb)
                            )

                        def dchain(qb=qb, h=h, ot_ps=ot_ps, accs=accs):
                            # D broadcast across partitions (each output row
                            # of ones^T @ es_sum is the key-dim column sum),
                            # fast reciprocal, O^T scale
                            d_ps = psD.tile([128, 512], F32, tag="dps",
                                            name="dps")
                            live = sorted((a for a in accs
                                           if a[1] is not None),
                                          key=lambda a: a[1])
                            for i, (t, o, _) in enumerate(live):
                                nc.tensor.matmul(
                                    d_ps[:, o:], ones_sb, t[:, o:],
                                    start=(i == 0), stop=(i == len(live) - 1),
                                )
                            rd = rdp.tile([128, 512], F32, tag="rd",
                                          name="rd")
                            nc.vector.reciprocal_approx_fast(out=rd, in_=d_ps)
                            ot = otp.tile([128, 512], BF16, tag=f"ot{h}",
                                          name=f"ot{h}")
                            nc.vector.tensor_mul(ot, ot_ps, rd)
                            ot_store[(qb, h)] = ot

                        deferred.append(dchain)
                        # held-back projection groups cover the deferred
                        # drain/D chain of this head
                        while c_done < len(citems):
                            citems[c_done]()
                            c_done += 1

                # trailing projection of the last query block
                flush()
                for qc in range(QH):
                    for it in c_items(NSB - 1, qc, last=True):
                        it()
    nc.finalize()
    return nc


_NC_CACHE = {}


def _get_nc():
    if "nc" not in _NC_CACHE:
        _NC_CACHE["nc"] = build_nc()
    return _NC_CACHE["nc"]


def _host_prep(x, cos, sin, mask, wq, wk, wv, wo):
    import ml_dtypes

    bf16 = ml_dtypes.bfloat16
    # partition-major shuffles: index [p, ...] with contraction tile t so
    # every DMA line is 4-8 KiB contiguous
    xS = np.ascontiguousarray(
        x[0].astype(bf16)                    # (S, D) = (sb*512+s, t*128+p)
        .reshape(NSB, 512, NKT, 128)
        .transpose(3, 0, 2, 1)               # (p, sb, t, s)
    )
    cosT = np.ascontiguousarray(cos[:, 0, :].T).astype(bf16)
    sinT = sin[:, 0, :].T.astype(np.float32)
    sinTs = np.ascontiguousarray(
        np.concatenate([-sinT[:64], sinT[64:]], axis=0)
    ).astype(bf16)
    rr = np.arange(128, dtype=np.int64)[:, None]
    cc = np.arange(128, dtype=np.int64)[None, :]
    tri = (rr <= cc).astype(np.float32).astype(bf16)
    ident = np.eye(128).astype(bf16)
    ones_mat = np.ones((128, 128), dtype=np.float32)

    def wshuf(w):
        # (t*128+p, m) -> (p, t*M+m)
        t = w.shape[0] // 128
        return np.ascontiguousarray(
            w.astype(bf16).reshape(t, 128, -1).transpose(1, 0, 2)
            .reshape(128, -1)
        )

    in_maps = []
    for i in range(N_CORES):
        in_maps.append({
            "xS": xS,
            "wqS": wshuf(wq[:, i * QS:(i + 1) * QS]),
            "wkS": wshuf(wk[:, i * 128:(i + 1) * 128]),
            "wvS": wshuf(wv[:, i * 128:(i + 1) * 128]),
            "woS": wshuf(wo[i * QS:(i + 1) * QS, :]),
            "cosT": cosT,
            "sinTs": sinTs,
            "tri": tri,
            "ident": ident,
            "ones_mat": ones_mat,
        })
    return in_maps


def kernel(x, cos, sin, mask, wq, wk, wv, wo, _trace=False, _trace_kwargs=None):
    nc = _get_nc()
    in_maps = _host_prep(x, cos, sin, mask, wq, wk, wv, wo)
    res = run_bass_kernel_spmd(
        nc, in_maps, list(range(N_CORES)), trace=_trace,
        **(_trace_kwargs or {}),
    )
    partials = [res.results[i]["out"] for i in range(N_CORES)]
    full = np.sum(
        np.stack([p.astype(np.float32) for p in partials], axis=0),
        axis=0, dtype=np.float64,
    )
    out = full.astype(np.float32)[None, :, :]
    if _trace:
        return out, res
    return out


# revision 36
# speedup vs baseline: 1.0111x; 1.0111x over previous
"""GQA attention (SEQ=2048, DIM=4096, 32 Q heads / 8 KV heads, head_dim=128),
tensor-parallel over heads across 8 NeuronCores.

Each core owns 4 Q heads + 1 KV head: wq/wk/wv split column-wise, wo split
row-wise; each core produces a partial (2048, 4096) output that the host sums
(the all-reduce of row-parallel wo).

Per-core kernel, bf16 matmul operands (1 cyc/row at any free size, half the
DMA/SBUF/DVE traffic of f32; PSUM accumulation stays fp32):
  A) QKV projections over per-chunk x tiles: an interleaved K||V pass tracks
     the x DMA stream, then Q0..Q3 passes re-read the resident chunks while
     RoPE / V-transpose work drains on ACT+DVE+PE underneath. Weight DMAs
     ride the (otherwise idle) GPSIMD queue so the x stream owns the sync
     queue's DMA engine.
  B/C) software-pipelined: the attention streams for query block qb carry
     the output projection of block qb-1 inside them. Per (head h, qb):
     S^T matmuls + exp (ACT, scale folded) + in-place 128x128 0/1 triangle
     mask on the diagonal (DVE) + row-sum accumulation (split GPSIMD/DVE)
     + lagged AV drains, with the wo-projection pair groups of (qb-1, qc=h)
     and the previous head's deferred tail drains / D-reciprocal chain
     interleaved between score matmuls. Diagonal blocks shrink their moving
     free dim to the causally-visible suffix. D is broadcast across
     partitions via a ones-matrix matmul, inverted with the fast custom-DVE
     reciprocal, and applied to O^T on DVE.
"""

import numpy as np

import concourse.bacc as bacc
import concourse.tile as tile
from concourse import mybir
from concourse.bass_utils import run_bass_kernel_spmd

F32 = mybir.dt.float32
F32R = mybir.dt.float32r
BF16 = mybir.dt.bfloat16

DIM = 4096
SEQ = 2048
HEAD_DIM = 128
N_CORES = 8
QH = 4              # q heads per core
QS = QH * HEAD_DIM  # 512: wq column slice per core
NKT = DIM // 128    # 32 contraction tiles
NSB = SEQ // 512    # 4 sequence blocks
SCALE = 1.0 / float(np.sqrt(HEAD_DIM))
LAG = 4             # AV matmuls trail the score stream by LAG blocks


def build_nc():
    nc = bacc.Bacc(trn_type="TRN2")

    # all big operands are pre-shuffled on the host into partition-major
    # layouts so every DMA moves 4-8 KiB contiguous lines per partition
    xS = nc.declare_dram_parameter("xS", [128, NSB, NKT, 512], BF16, isOutput=False)
    wqS = nc.declare_dram_parameter("wqS", [128, NKT * QS], BF16, isOutput=False)
    wkS = nc.declare_dram_parameter("wkS", [128, NKT * HEAD_DIM], BF16, isOutput=False)
    wvS = nc.declare_dram_parameter("wvS", [128, NKT * HEAD_DIM], BF16, isOutput=False)
    woS = nc.declare_dram_parameter("woS", [128, QH * DIM], BF16, isOutput=False)
    cosT = nc.declare_dram_parameter("cosT", [HEAD_DIM, SEQ], BF16, isOutput=False)
    sinTs = nc.declare_dram_parameter("sinTs", [HEAD_DIM, SEQ], BF16, isOutput=False)
    tri = nc.declare_dram_parameter("tri", [128, 128], BF16, isOutput=False)
    ident = nc.declare_dram_parameter("ident", [128, 128], BF16, isOutput=False)
    ones_mat = nc.declare_dram_parameter("ones_mat", [128, 128], F32R, isOutput=False)
    out = nc.declare_dram_parameter("out", [SEQ, DIM], BF16, isOutput=True)

    with tile.TileContext(nc) as tc:
        with (
            tc.tile_pool(name="persist", bufs=1) as persist,
            tc.tile_pool(name="resid", bufs=1) as resid,
        ):
            tri_sb = persist.tile([128, 128], BF16)
            ident_sb = persist.tile([128, 128], BF16)
            ones_sb = persist.tile([128, 128], F32R)

            # resident activations; qT split per head so phase B's first
            # score matmuls only wait on their own head's RoPE
            qTs = [resid.tile([128, SEQ], BF16, name=f"qT{h}")
                   for h in range(QH)]
            kT = resid.tile([128, SEQ], BF16)              # K^T (d, s)
            vN = resid.tile([128, SEQ // 128, 128], BF16)  # V natural (k, d)

            # ---------------- Phase A: projections + RoPE ----------------
            with (
                tc.tile_pool(name="wpool", bufs=1) as wpool,
                tc.tile_pool(name="xpool", bufs=2) as xpool,
                tc.tile_pool(name="cspool", bufs=2) as cspool,
                tc.tile_pool(name="ropetmp", bufs=2) as ropetmp,
                tc.tile_pool(name="vtb", bufs=2) as vtb,
                tc.tile_pool(name="psA", bufs=1, space="PSUM") as psA,
                tc.tile_pool(name="psVT", bufs=2, space="PSUM") as psVT,
            ):
                # per-chunk weight tiles: a matmul only waits on the one DMA
                # that feeds its chunk, not the whole weight load. Weights
                # ride the scalar HW-DGE queue, x owns the sync queue.
                wk_cs = [wpool.tile([128, 8 * HEAD_DIM], BF16, name=f"wk{c}")
                         for c in range(4)]
                wv_cs = [wpool.tile([128, 8 * HEAD_DIM], BF16, name=f"wv{c}")
                         for c in range(4)]
                wq_cs = [wpool.tile([128, 4 * QS], BF16, name=f"wq{c}")
                         for c in range(8)]

                def xchunks(sb):
                    xs = []
                    for g in range(8):
                        xg = xpool.tile([128, 4, 512], BF16, tag=f"xb{g}",
                                        name=f"xb{g}")
                        nc.sync.dma_start(out=xg, in_=xS[:, sb, g * 4:(g + 1) * 4, :])
                        xs.append(xg)
                    return xs

                def wdma(c, q):
                    nc.scalar.dma_start(
                        out=wk_cs[c], in_=wkS[:, c * 1024:(c + 1) * 1024])
                    nc.scalar.dma_start(
                        out=wv_cs[c], in_=wvS[:, c * 1024:(c + 1) * 1024])
                    for i in range(2):
                        if 2 * c + i in q:
                            nc.scalar.dma_start(
                                out=wq_cs[2 * c + i],
                                in_=wqS[:, (2 * c + i) * 2048:
                                        (2 * c + i + 1) * 2048])

                # scalar queue in exact consumption order; the last wq
                # chunks ride the sync queue once the x stream is done
                wdma(0, (0, 1))
                xb = xchunks(0)
                wdma(1, (2, 3))
                cos_t = cspool.tile([128, 512], BF16, tag="cos")
                nc.scalar.dma_start(out=cos_t, in_=cosT[:, 0:512])
                sin_t = cspool.tile([128, 512], BF16, tag="sin")
                nc.scalar.dma_start(out=sin_t, in_=sinTs[:, 0:512])
                wdma(2, (4,))
                wdma(3, ())
                for i in (5, 6, 7):
                    nc.sync.dma_start(
                        out=wq_cs[i],
                        in_=wqS[:, i * 2048:(i + 1) * 2048])

                # small constants ride behind the critical weight DMAs
                nc.scalar.dma_start(out=ident_sb, in_=ident[:, :])
                nc.scalar.dma_start(out=tri_sb, in_=tri[:, :])
                nc.scalar.dma_start(out=ones_sb, in_=ones_mat[:, :])

                def rope(dst, src_ps, cos_t, sin_t):
                    # ACT copies evict PSUM fast and produce the straight and
                    # half-rotated views (partition-shifted reads are only
                    # legal on ACT); DVE runs the bf16 mul/mul/add at 2x rate.
                    v = ropetmp.tile([128, 512], BF16, tag="v", name="v")
                    vr = ropetmp.tile([128, 512], BF16, tag="vr", name="vr")
                    nc.scalar.copy(v, src_ps)
                    nc.scalar.copy(vr[0:64, :], src_ps[64:128, :])
                    nc.scalar.copy(vr[64:128, :], src_ps[0:64, :])
                    t = ropetmp.tile([128, 512], BF16, tag="t", name="t")
                    u = ropetmp.tile([128, 512], BF16, tag="u", name="u")
                    nc.vector.tensor_mul(t, v, cos_t)
                    nc.vector.tensor_mul(u, vr, sin_t)
                    nc.vector.tensor_add(dst, t, u)

                for sb in range(NSB):
                    ss = slice(sb * 512, (sb + 1) * 512)
                    if sb > 0:
                        xb = xchunks(sb)
                        cos_t = cspool.tile([128, 512], BF16, tag="cos")
                        nc.scalar.dma_start(out=cos_t, in_=cosT[:, ss])
                        sin_t = cspool.tile([128, 512], BF16, tag="sin")
                        nc.scalar.dma_start(out=sin_t, in_=sinTs[:, ss])

                    # interleaved K||V(||Q0 for the cold-start block)
                    # pass: the PE consumes each x chunk at the DMA delivery
                    # rate; rope-K and the V evict then run on ACT/DVE under
                    # the remaining Q passes
                    def qmm(q_ps, h, kt):
                        nc.tensor.matmul(
                            q_ps,
                            wq_cs[kt // 4][:, (kt % 4) * 512 + h * 128:
                                           (kt % 4) * 512 + (h + 1) * 128],
                            xb[kt // 4][:, kt % 4, :],
                            start=(kt == 0), stop=(kt == NKT - 1),
                        )

                    k_ps = psA.tile([128, 512], F32, tag="kps", name="kps")
                    v_ps = psA.tile([128, 512], F32, tag="vps", name="vps")
                    q_ps0 = psA.tile([128, 512], F32, tag="qps0", name="qps0")

                    def kmm(kt):
                        nc.tensor.matmul(
                            k_ps,
                            wk_cs[kt // 8][:, (kt % 8) * 128:
                                           (kt % 8 + 1) * 128],
                            xb[kt // 4][:, kt % 4, :],
                            start=(kt == 0), stop=(kt == NKT - 1),
                        )

                    def vmm(kt):
                        nc.tensor.matmul(
                            v_ps,
                            wv_cs[kt // 8][:, (kt % 8) * 128:
                                           (kt % 8 + 1) * 128],
                            xb[kt // 4][:, kt % 4, :],
                            start=(kt == 0), stop=(kt == NKT - 1),
                        )

                    for kt in range(NKT):
                        kmm(kt)
                        vmm(kt)
                        if sb == 0:
                            qmm(q_ps0, 0, kt)
                    rope(kT[:, ss], k_ps, cos_t, sin_t)
                    vt_sb = vtb.tile([128, 512], BF16, tag="vt", name="vt")
                    nc.scalar.copy(vt_sb, v_ps)
                    if sb == 0:
                        rope(qTs[0][:, ss], q_ps0, cos_t, sin_t)

                    for h in range(QH):
                        if h == 0:
                            if sb == 0:
                                continue
                            q_ps = q_ps0
                        else:
                            q_ps = psA.tile([128, 512], F32, tag=f"qps{h}",
                                            name=f"qps{h}")
                        for kt in range(NKT):
                            qmm(q_ps, h, kt)
                        if h == 1:
                            # PE transposes of V^T -> V, slotted between Q
                            # passes so they never wait on the ACT evict
                            for j in range(4):
                                vt_ps = psVT.tile([128, 128], BF16, tag="vtp",
                                                  name="vtp")
                                nc.tensor.transpose(
                                    vt_ps, vt_sb[:, j * 128:(j + 1) * 128],
                                    ident_sb,
                                )
                                nc.scalar.copy(vN[:, sb * 4 + j, :], vt_ps)
                        rope(qTs[h][:, ss], q_ps, cos_t, sin_t)

            # ------- Phase B/C: attention with pipelined out projection -------
            with (
                tc.tile_pool(name="wopool", bufs=1) as wopool,
                tc.tile_pool(name="expp", bufs=24) as expp,
                tc.tile_pool(name="esum", bufs=2) as esum,
                tc.tile_pool(name="rdp", bufs=2) as rdp,
                tc.tile_pool(name="otp", bufs=2) as otp,
                tc.tile_pool(name="stg", bufs=2) as stg,
                # declaration order maps pools onto the banks phase A frees
                # first (k/v accumulators release during the Q passes; the
                # q accumulators only after their trailing RoPE)
                tc.tile_pool(name="psOT", bufs=2, space="PSUM") as psOT,
                tc.tile_pool(name="psS", bufs=2, space="PSUM") as psS,
                tc.tile_pool(name="psD", bufs=2, space="PSUM") as psD,
                tc.tile_pool(name="psC", bufs=2, space="PSUM") as psC,
            ):
                wo_sbs = [wopool.tile([128, DIM], BF16, name=f"wo{h}")
                          for h in range(QH)]
                for h in range(QH):
                    nc.scalar.dma_start(
                        out=wo_sbs[h], in_=woS[:, h * DIM:(h + 1) * DIM]
                    )

                ot_store = {}   # (qb, h) -> normalized O^T tile
                deferred = []   # closures: tail drains + D chains, issued
                                # under later PE work

                def flush(n=None):
                    k = len(deferred) if n is None else min(n, len(deferred))
                    for _ in range(k):
                        deferred.pop(0)()

                def c_items(qb, qc, last=False):
                    """Output-projection work of query block qb, row stripe
                    qc: 8 accumulation groups + evicts + 2 half-stripe DMAs,
                    as a list of closures."""
                    stg_t = stg.tile([128, DIM], BF16, tag="stg", name="stg")
                    items = []

                    def group(nb, evict_dve):
                        def go():
                            o_ps = psC.tile([128, 512], F32, tag="ops",
                                            name="ops")
                            for h in range(QH):
                                nc.tensor.matmul(
                                    o_ps,
                                    ot_store[(qb, h)][:, qc * 128:
                                                      (qc + 1) * 128],
                                    wo_sbs[h][:, nb * 512:(nb + 1) * 512],
                                    start=(h == 0), stop=(h == QH - 1),
                                )
                            dst = stg_t[:, nb * 512:(nb + 1) * 512]
                            if evict_dve:
                                nc.vector.tensor_copy(dst, o_ps)
                            else:
                                nc.scalar.copy(dst, o_ps)
                            if last:
                                nc.sync.dma_start(
                                    out=out[qb * 512 + qc * 128:
                                            qb * 512 + (qc + 1) * 128,
                                            nb * 512:(nb + 1) * 512],
                                    in_=dst,
                                )
                        return go

                    def dma(lo, hi):
                        def go():
                            nc.sync.dma_start(
                                out=out[qb * 512 + qc * 128:
                                        qb * 512 + (qc + 1) * 128,
                                        lo * 512:hi * 512],
                                in_=stg_t[:, lo * 512:hi * 512],
                            )
                        return go

                    for nb in range(8):
                        # during-stream evicts lean on DVE (ACT is running
                        # the exp stream); the trailing block alternates
                        items.append(group(nb, evict_dve=(nb < 5) if not last
                                           else nb % 2 == 0))
                        if not last:
                            if nb == 3:
                                items.append(dma(0, 4))
                            elif nb == 7:
                                items.append(dma(4, 8))
                    return items

                for qb in range(NSB):
                    qs = slice(qb * 512, (qb + 1) * 512)
                    n_kb = 4 * qb + 4
                    for h in range(QH):
                        citems = c_items(qb - 1, h) if qb > 0 else []
                        c_done = 0
                        ot_ps = psOT.tile([128, 512], F32, tag="otps",
                                          name="otps")
                        # row-sum accumulators: [tile, start offset or None,
                        # engine] -- one fed by GPSIMD, one by DVE
                        accs = [
                            [esum.tile([128, 512], F32R, tag="esa",
                                       name="esa"), None,
                             nc.vector if h == QH - 1 else nc.gpsimd],
                            [esum.tile([128, 512], F32R, tag="esb",
                                       name="esb"), None, nc.vector],
                        ]
                        nacc = [0]
                        pend = [None]

                        def acc_push(t, off):
                            a = accs[nacc[0] % 2]
                            nacc[0] += 1
                            if a[1] is None:
                                a[2].tensor_copy(a[0][:, off:], t[:, off:])
                                a[1] = off
                            else:
                                a[2].tensor_add(a[0][:, off:],
                                                a[0][:, off:], t[:, off:])
                        ess = [None] * n_kb
                        offs = [max(0, kb - 4 * qb) * 128
                                for kb in range(n_kb)]

                        def drain(kb, ot_ps=ot_ps, ess=ess, n_kb=n_kb,
                                  offs=offs):
                            o = offs[kb]
                            nc.tensor.matmul(
                                ot_ps[:, o:], vN[:, kb, :], ess[kb][:, o:],
                                start=(kb == 0), stop=(kb == n_kb - 1),
                            )

                        for kb in range(n_kb):
                            off = offs[kb]
                            s_ps = psS.tile([128, 512], F32, tag="sps",
                                            name="sps")
                            nc.tensor.matmul(
                                s_ps[:, off:],
                                kT[:, kb * 128:(kb + 1) * 128],
                                qTs[h][:, qb * 512 + off:(qb + 1) * 512],
                                start=True, stop=True,
                            )
                            es = expp.tile([128, 512], BF16, tag="es",
                                           name="es")
                            nc.scalar.activation(
                                es[:, off:], s_ps[:, off:],
                                mybir.ActivationFunctionType.Exp,
                                scale=SCALE,
                            )
                            if kb - 4 * qb >= 0:
                                # in-place 0/1 lower-triangle mask on the
                                # 128-wide diagonal sub-block
                                nc.vector.tensor_mul(
                                    es[:, off:off + 128],
                                    es[:, off:off + 128], tri_sb,
                                )
                            ess[kb] = es
                            # row-sum: full-width blocks reduce pairwise in
                            # bf16 on the DVE 2x path first (error averages
                            # out over the pair), then the f32 accumulators
                            # alternate GPSIMD/DVE
                            if off == 0 and kb < 4 * qb:
                                if pend[0] is None:
                                    pend[0] = es
                                else:
                                    esp = expp.tile([128, 512], BF16,
                                                    tag="es", name="esp")
                                    nc.vector.tensor_add(esp, pend[0], es)
                                    pend[0] = None
                                    acc_push(esp, 0)
                            else:
                                acc_push(es, off)
                            if kb >= LAG:
                                drain(kb - LAG)
                            # previous head's deferred chain, then this
                            # head's share of the qb-1 projection work
                            if kb == 0:
                                flush(2)
                            elif kb == 1:
                                flush(2)
                            elif kb == 2:
                                flush()
                            elif citems:
                                hold = (8 if qb == NSB - 1 else 5
                                        ) if h == QH - 1 else 0
                                want = ((len(citems) - hold) * (kb - 2)
                                        ) // (n_kb - 3)
                                while c_done < want:
                                    citems[c_done]()
                                    c_done += 1
                        for kb in range(max(0, n_kb - LAG), n_kb):
                            deferred.append(
                                lambda kb=kb, drain=drain: drain(kb)
                            )

                        def dchain(qb=qb, h=h, ot_ps=ot_ps, accs=accs):
                            # D broadcast across partitions (each output row
                            # of ones^T @ es_sum is the key-dim column sum),
                            # fast reciprocal, O^T scale
                            d_ps = psD.tile([128, 512], F32, tag="dps",
                                            name="dps")
                            live = sorted((a for a in accs
                                           if a[1] is not None),
                                          key=lambda a: a[1])
                            for i, (t, o, _) in enumerate(live):
                                nc.tensor.matmul(
                                    d_ps[:, o:], ones_sb, t[:, o:],
                                    start=(i == 0), stop=(i == len(live) - 1),
                                )
                            rd = rdp.tile([128, 512], F32, tag="rd",
                                          name="rd")
                            nc.vector.reciprocal_approx_fast(out=rd, in_=d_ps)
                            ot = otp.tile([128, 512], BF16, tag=f"ot{h}",
                                          name=f"ot{h}")
                            nc.vector.tensor_mul(ot, ot_ps, rd)
                            ot_store[(qb, h)] = ot

                        deferred.append(dchain)
                        # held-back projection groups cover the deferred
                        # drain/D chain of this head
                        while c_done < len(citems):
                            citems[c_done]()
                            c_done += 1

                # trailing projection of the last query block
                flush()
                for qc in range(QH):
                    for it in c_items(NSB - 1, qc, last=True):
                        it()
    nc.finalize()
    return nc


_NC_CACHE = {}


def _get_nc():
    if "nc" not in _NC_CACHE:
        _NC_CACHE["nc"] = build_nc()
    return _NC_CACHE["nc"]


def _host_prep(x, cos, sin, mask, wq, wk, wv, wo):
    import ml_dtypes

    bf16 = ml_dtypes.bfloat16
    # partition-major shuffles: index [p, ...] with contraction tile t so
    # every DMA line is 4-8 KiB contiguous
    xS = np.ascontiguousarray(
        x[0].astype(bf16)                    # (S, D) = (sb*512+s, t*128+p)
        .reshape(NSB, 512, NKT, 128)
        .transpose(3, 0, 2, 1)               # (p, sb, t, s)
    )
    cosT = np.ascontiguousarray(cos[:, 0, :].T).astype(bf16)
    sinT = sin[:, 0, :].T.astype(np.float32)
    sinTs = np.ascontiguousarray(
        np.concatenate([-sinT[:64], sinT[64:]], axis=0)
    ).astype(bf16)
    rr = np.arange(128, dtype=np.int64)[:, None]
    cc = np.arange(128, dtype=np.int64)[None, :]
    tri = (rr <= cc).astype(np.float32).astype(bf16)
    ident = np.eye(128).astype(bf16)
    ones_mat = np.ones((128, 128), dtype=np.float32)

    def wshuf(w):
        # (t*128+p, m) -> (p, t*M+m)
        t = w.shape[0] // 128
        return np.ascontiguousarray(
            w.astype(bf16).reshape(t, 128, -1).transpose(1, 0, 2)
            .reshape(128, -1)
        )

    in_maps = []
    for i in range(N_CORES):
        in_maps.append({
            "xS": xS,
            "wqS": wshuf(wq[:, i * QS:(i + 1) * QS]),
            "wkS": wshuf(wk[:, i * 128:(i + 1) * 128]),
            "wvS": wshuf(wv[:, i * 128:(i + 1) * 128]),
            "woS": wshuf(wo[i * QS:(i + 1) * QS, :]),
            "cosT": cosT,
            "sinTs": sinTs,
            "tri": tri,
            "ident": ident,
            "ones_mat": ones_mat,
        })
    return in_maps


def kernel(x, cos, sin, mask, wq, wk, wv, wo, _trace=False, _trace_kwargs=None):
    nc = _get_nc()
    in_maps = _host_prep(x, cos, sin, mask, wq, wk, wv, wo)
    res = run_bass_kernel_spmd(
        nc, in_maps, list(range(N_CORES)), trace=_trace,
        **(_trace_kwargs or {}),
    )
    partials = [res.results[i]["out"] for i in range(N_CORES)]
    full = np.sum(
        np.stack([p.astype(np.float32) for p in partials], axis=0),
        axis=0, dtype=np.float64,
    )
    out = full.astype(np.float32)[None, :, :]
    if _trace:
        return out, res
    return out
